# revision 66
# baseline (speedup 1.0000x reference)
"""AttnRNN decoder kernel for trn2 (8 NeuronCores, data-parallel over batch).

Column-form ("transposed") formulation: every per-step matmul keeps the
large dimension on PSUM partitions via stationary weight tiles and streams
only the per-core batch (BS=4) or a single column as the moving operand.

  host   : embedding gather, weight transposes + scale folding
           (h stored as 2h; W_hh / M2 / out_w prescaled by 0.5;
            i,f,o gate columns prescaled by 0.5 for the
            sigmoid(x) = 0.5*tanh(x/2)+0.5 identity)
  device : phase0  seq_q / M2 / key_enc / s0 (t-independent score half,
                   precomputed for all 127 steps) in column layouts
           phase1  127 sequential steps; only Exp/Tanh/Relu activations
                   (single act table, zero mid-loop table reloads)
           phase2  vocab projection batched (4 ex x 32 steps = 128 psum
                   partitions), log_softmax as x - ln(sum exp x) with
                   per-partition scalar subtract; interleaved into the
                   tail of phase 1 in 32-step chunks
"""

import numpy as np

import concourse.bass as bass
import concourse.bacc as bacc
import concourse.mybir as mybir
import concourse.tile as tile
from concourse.bass_utils import run_bass_kernel_spmd

F32 = mybir.dt.float32
BF16 = mybir.dt.bfloat16
AF = mybir.ActivationFunctionType
ALU = mybir.AluOpType

B, L, D, T = 32, 512, 256, 128
H, NCOM, NB = 256, 8000, 128
V = NCOM + NB
NCORES = 8
BS = B // NCORES          # 4 examples per core
TT = T - 1                # 127 decode steps
TP = 128                  # padded steps (col 127 of linT zeroed)

# vocab chunking for phase 2: 15x512 + 1x320 common, then 128 batched
VCH = [(i * 512, 512) for i in range(15)] + [(7680, 320)]


def _pmajor(x, nchunk):
    """(nchunk*128, ...) -> (128, nchunk, ...) partition-major."""
    s = x.shape
    return np.ascontiguousarray(
        x.reshape(nchunk, 128, *s[1:]).transpose(1, 0, *range(2, 1 + len(s)))
    )


def _build(cfg):
    """Build the single-core program (SPMD-replicated across 8 cores)."""
    nc = bacc.Bacc("TRN2", target_bir_lowering=False, debug=False)

    dr = {}

    def din(name, shape, dt=F32):
        dr[name] = nc.dram_tensor(name, list(shape), dt, kind="ExternalInput").ap()
        return dr[name]

    enct = din("enct", (128, 2, BS, L), BF16)      # enc[b, l, 128*dc+p]
    tgtt = din("tgtt", (128, 2, TT, BS), BF16)     # tgt[b, t, 128*dc+p]
    cembt = din("cembt", (128, 2, NCOM), BF16)     # common[v, 128*dc+p]
    obt = din("obt", (128, 2, BS, NB), BF16)       # batched[b, v, 128*dc+p]
    wket = din("wket", (128, 2, D), BF16)
    wqe = din("wqe", (128, 2, D), BF16)     # Wq[e, d] with e on partitions
    m2w = din("m2w", (128, 2, D), BF16)     # 0.5*Wq.T@akw_h [d, f], d on parts
    if cfg["any_bias"]:
        wqt = din("wqt", (128, 2, D), BF16)        # Wq[e, 128*dc+p]
        wkh2 = din("wkh2", (128, 2, D), BF16)   # 0.5*akw[e, 256+f]
    cwt = din("cwt", (128, 4, H), BF16)            # combine_w.T chunks
    w2d = din("w2d", (128, 2, H), BF16)            # W2.T [d, h], d on parts
    lwt = din("lwt", (128, 4, 4 * H), BF16)        # [W_ih.T; 0.5*W_hh.T] col-scaled
    owt = din("owt", (128, 2, D), BF16)            # 0.25*out_w.T (2h in, /2 out? see host)
    id32 = din("id32", (32, 32), BF16)             # one-hot columns for s0 inject
    ones128 = din("ones128", (128, 128), BF16)     # Z partition-sum stationary
    if cfg["any_bias"]:
        bqc = din("bqc", (128, 2))
        bkc = din("bkc", (128, 2))
        # brow: [cb(256) | lstm bias col-scaled (1024) | 0.5*out_b (256)]
        brow = din("brow", (1, 2 * H + 4 * H + D), BF16)
        onesb = din("onesb", (1, BS), BF16)
    if cfg["enc_mask"]:
        em_r = din("em_r", (BS, L), BF16)          # 0 / -1e30 row per example
        ones127 = din("ones127", (1, TT), BF16)
    if cfg["out_mask"]:
        bmr = din("bmr", (BS, NB), BF16)
        onest = din("onest", (1, TP), BF16)

    out = nc.dram_tensor("out", [BS, TT, V], F32, kind="ExternalOutput").ap()

    with tile.TileContext(nc) as tc:
        with (
            tc.tile_pool(name="const", bufs=1) as kc,
            tc.tile_pool(name="state", bufs=3) as stp,
        ):
            # ---- persistent SBUF loads ----
            # phase-0-critical tensors first (smallest-to-largest on the
            # serial DMA issue path): weights, tgtt, then enct
            tgtt_sb = kc.tile([128, 2, TT, BS], BF16)
            nc.sync.dma_start(tgtt_sb[:], tgtt[:])
            wket_sb = kc.tile([128, 2, D], BF16)
            nc.sync.dma_start(wket_sb[:], wket[:])
            wqe_sb = kc.tile([128, 2, D], BF16)
            nc.sync.dma_start(wqe_sb[:], wqe[:])
            m2w_sb = kc.tile([128, 2, D], BF16)
            nc.sync.dma_start(m2w_sb[:], m2w[:])
            enct_sb0 = kc.tile([128, 2, BS, L], BF16, name="enct_sb0")
            nc.sync.dma_start(enct_sb0[:], enct[:])
            if cfg["any_bias"]:
                wqt_sb = kc.tile([128, 2, D], BF16)
                nc.sync.dma_start(wqt_sb[:], wqt[:])
                wkh2_sb = kc.tile([128, 2, D], BF16)
                nc.sync.dma_start(wkh2_sb[:], wkh2[:])
            cwt_sb = kc.tile([128, 4, H], BF16)
            nc.sync.dma_start(cwt_sb[:], cwt[:])
            w2d_sb = kc.tile([128, 2, H], BF16)
            nc.sync.dma_start(w2d_sb[:], w2d[:])
            lwt_sb = kc.tile([128, 4, 4 * H], BF16)
            nc.sync.dma_start(lwt_sb[:], lwt[:])
            owt_sb = kc.tile([128, 2, D], BF16)
            nc.sync.dma_start(owt_sb[:], owt[:])
            id32_sb = kc.tile([32, 32], BF16)
            nc.sync.dma_start(id32_sb[:], id32[:])
            ones_sb = kc.tile([128, 128], BF16)
            nc.sync.dma_start(ones_sb[:], ones128[:])
            if cfg["any_bias"]:
                bqc_sb = kc.tile([128, 2], F32)
                nc.sync.dma_start(bqc_sb[:], bqc[:])
                bkc_sb = kc.tile([128, 2], F32)
                nc.sync.dma_start(bkc_sb[:], bkc[:])
                brow_sb = kc.tile([1, 2 * H + 4 * H + D], BF16)
                nc.sync.dma_start(brow_sb[:], brow[:])
                onesb_sb = kc.tile([1, BS], BF16)
                nc.sync.dma_start(onesb_sb[:], onesb[:])
            if cfg["enc_mask"]:
                em_sb = kc.tile([BS, L], BF16)
                nc.sync.dma_start(em_sb[:], em_r[:])
                ones127_sb = kc.tile([1, TT], BF16)
                nc.sync.dma_start(ones127_sb[:], ones127[:])
            if cfg["out_mask"]:
                bmr_sb = kc.tile([BS, NB], BF16)
                nc.sync.dma_start(bmr_sb[:], bmr[:])
                onest_sb = kc.tile([1, TP], BF16)
                nc.sync.dma_start(onest_sb[:], onest[:])

            # phase-2-only tensors: DMAs issued after phase-0 emission
            cembt_sb = kc.tile([128, 2, NCOM], BF16)
            obt_sb = kc.tile([128, 2, BS, NB], BF16)

            if cfg["any_bias"]:
                seqqt_sb = kc.tile([128, 2, BS, L], BF16, name="seqqt_sb")
            m2t_sb = kc.tile([128, 2, BS, L], BF16)
            ew_sb = kc.tile([128, BS, 4, H], BF16)   # EW[b][l,h] = enc@W2.T
            ket_sb = kc.tile([128, 2, TT, BS], BF16)
            # s0 in one-hot-injectable layout: [32 tr, b, lc, tg, 128 l]
            s032_sb = kc.tile([32, BS, 4, 4, 128], BF16)
            # t=127 doesn't exist: zero the tg=3 slab so row tr=31 never
            # pulls uninitialized data into the PE array (partition starts
            # must be 32-aligned, so clear the whole slab; the phase-0
            # copies overwrite rows 0..30 afterwards)
            nc.gpsimd.memset(s032_sb[:, :, :, 3, :], 0.0)
            # lin in stationary layout for phase 2, per-t-chunk contiguous
            # blocks so the (b, t) stationary AP collapses to one free dim:
            # [128 d, dc, tc, b, tr]; chunk 3 covers t=95..126 (row t=95 is
            # double-written into chunks 2 and 3)
            linT_sb = kc.tile([128, 2, 4, BS, 32], BF16)

            # ---- phase 0 ----
            with (
                tc.tile_pool(name="p0ps", bufs=2, space="PSUM") as p0,
                tc.tile_pool(name="p0s0", bufs=2, space="PSUM") as p0s,
            ):
                # key_enc (with key bias): [e-part, ec, t, b]
                for c in range(2):
                    ps = p0.tile([128, 512], F32, tag="p0", name="p0")
                    for k in range(2):
                        nc.tensor.matmul(
                            ps[:, 0:TT * BS],
                            wket_sb[:, k, c * 128:(c + 1) * 128],
                            tgtt_sb[:, k, :, :],
                            start=(k == 0), stop=(k == 1),
                        )
                    if cfg["any_bias"]:
                        nc.scalar.activation(
                            ket_sb[:, c, :, :], ps[:, 0:TT * BS], AF.Identity,
                            bias=bkc_sb[:, c:c + 1],
                        )
                    else:
                        nc.vector.tensor_copy(ket_sb[:, c, :, :], ps[:, 0:TT * BS])

                if cfg["any_bias"]:
                    # slow path: materialize seq_q (incl. query bias), then
                    # M2 = 0.5*akw_h^T seqq and s0 = ket^T seqq
                    for b in range(BS):
                        for c in range(2):
                            ps = p0.tile([128, 512], F32, tag="p0", name="p0")
                            for k in range(2):
                                nc.tensor.matmul(
                                    ps[:],
                                    wqt_sb[:, k, c * 128:(c + 1) * 128],
                                    enct_sb0[:, k, b, :],
                                    start=(k == 0), stop=(k == 1),
                                )
                            nc.scalar.activation(
                                seqqt_sb[:, c, b, :], ps[:], AF.Identity,
                                bias=bqc_sb[:, c:c + 1],
                            )
                    for b in range(BS):
                        ps = p0s.tile([TT, 512], F32, tag="s0", name="s0")
                        nmm = 2 + (1 if cfg["enc_mask"] else 0)
                        for k in range(2):
                            nc.tensor.matmul(
                                ps[:],
                                ket_sb[:, k, :, b],
                                seqqt_sb[:, k, b, :],
                                start=(k == 0), stop=(k == nmm - 1),
                            )
                        if cfg["enc_mask"]:
                            nc.tensor.matmul(
                                ps[:], ones127_sb[0:1, :], em_sb[b:b + 1, :],
                                start=False, stop=True,
                            )
                        for tg in range(4):
                            nt = min(32, TT - tg * 32)
                            if tg % 2 == 1:
                                nc.scalar.activation(
                                    s032_sb[0:nt, b, :, tg, :],
                                    ps[tg * 32:tg * 32 + nt, :], AF.Identity)
                            else:
                                eng.tensor_copy(
                                    s032_sb[0:nt, b, :, tg, :],
                                    ps[tg * 32:tg * 32 + nt, :])
                    for b in range(BS):
                        for c in range(2):
                            ps = p0.tile([128, 512], F32, tag="p0", name="p0")
                            for k in range(2):
                                nc.tensor.matmul(
                                    ps[:],
                                    wkh2_sb[:, k, c * 128:(c + 1) * 128],
                                    seqqt_sb[:, k, b, :],
                                    start=(k == 0), stop=(k == 1),
                                )
                            nc.vector.tensor_copy(m2t_sb[:, c, b, :], ps[:])
                    for b in range(BS):
                        for lc in range(4):
                            ps = p0.tile([128, 512], F32, tag="p0", name="p0")
                            for k in range(2):
                                nc.tensor.matmul(
                                    ps[:, 0:H],
                                    enct_sb0[:, k, b, lc * 128:(lc + 1) * 128],
                                    w2d_sb[:, k, :],
                                    start=(k == 0), stop=(k == 1),
                                )
                            nc.vector.tensor_copy(
                                ew_sb[:, b, lc, :], ps[:, 0:H])
                else:
                    # fast path: fold Wq through on the host.
                    # kq[d,t,b] = sum_e Wq[e,d] ket[e,t,b]
                    kq_sb = kc.tile([128, 2, TT, BS], BF16, name="kq_sb")
                    for c in range(2):
                        ps = p0.tile([128, 512], F32, tag="p0", name="p0")
                        for k in range(2):
                            nc.tensor.matmul(
                                ps[:, 0:TT * BS],
                                wqe_sb[:, k, c * 128:(c + 1) * 128],
                                ket_sb[:, k, :, :],
                                start=(k == 0), stop=(k == 1),
                            )
                        if c == 0:
                            nc.vector.tensor_copy(
                                kq_sb[:, c, :, :], ps[:, 0:TT * BS])
                        else:
                            nc.scalar.activation(
                                kq_sb[:, c, :, :], ps[:, 0:TT * BS],
                                AF.Identity)
                    # s0[b][t,l] = sum_d kq[d,t,b] * enc[b,l,d]  (+ enc mask)
                    s0ps = []
                    for b in range(BS):
                        ps = p0s.tile([TT, 512], F32, tag=f"s0{b}",
                                      name="s0ps_t", bufs=1)
                        s0ps.append(ps)
                        nmm = 2 + (1 if cfg["enc_mask"] else 0)
                        for k in range(2):
                            nc.tensor.matmul(
                                ps[:],
                                kq_sb[:, k, :, b],
                                enct_sb0[:, k, b, :],
                                start=(k == 0), stop=(k == nmm - 1),
                            )
                        if cfg["enc_mask"]:
                            nc.tensor.matmul(
                                ps[:], ones127_sb[0:1, :], em_sb[b:b + 1, :],
                                start=False, stop=True,
                            )
                        # tg=0 copy immediately: step 0 needs only these
                        # rows, so they must not queue behind tg1-3 copies
                        if b % 2 == 0:
                            nc.vector.tensor_copy(
                                s032_sb[0:32, b, :, 0, :], ps[0:32, :])
                        else:
                            nc.scalar.activation(
                                s032_sb[0:32, b, :, 0, :], ps[0:32, :],
                                AF.Identity)
                    # EW first: step 0 needs it right after its exp; the
                    # tg1-3 s0 rows are only read from step 32 on
                    for b in range(BS):
                        for lc in range(4):
                            ps = p0.tile([128, 512], F32, tag="p0", name="p0")
                            for k in range(2):
                                nc.tensor.matmul(
                                    ps[:, 0:H],
                                    enct_sb0[:, k, b, lc * 128:(lc + 1) * 128],
                                    w2d_sb[:, k, :],
                                    start=(k == 0), stop=(k == 1),
                                )
                            if (b + lc) % 2 == 0:
                                nc.vector.tensor_copy(
                                    ew_sb[:, b, lc, :], ps[:, 0:H])
                            else:
                                nc.scalar.activation(
                                    ew_sb[:, b, lc, :], ps[:, 0:H],
                                    AF.Identity)
                    # remaining s0 rows after the EW copies are enqueued
                    for b in range(BS):
                        ps = s0ps[b]
                        for tg in range(1, 4):
                            nt = min(32, TT - tg * 32)
                            if (tg + b) % 2 == 1:
                                nc.scalar.activation(
                                    s032_sb[0:nt, b, :, tg, :],
                                    ps[tg * 32:tg * 32 + nt, :], AF.Identity)
                            else:
                                nc.vector.tensor_copy(
                                    s032_sb[0:nt, b, :, tg, :],
                                    ps[tg * 32:tg * 32 + nt, :])

            # phase-2-only input DMAs (needed from t>=33; don't block startup)
            nc.sync.dma_start(cembt_sb[:], cembt[:])
            nc.sync.dma_start(obt_sb[:], obt[:])

            # ---- phase 1: 127 sequential steps, column form ----
            # ---- phase 2: interleaved in 32-step chunks ----
            NCH = len(VCH) + 1  # 16 common chunks + 1 batched

            with (
                tc.tile_pool(name="sps", bufs=2, space="PSUM") as sps,
                tc.tile_pool(name="zps", bufs=2, space="PSUM") as zps,
                tc.tile_pool(name="p2a", bufs=4, space="PSUM") as p2a,
                tc.tile_pool(name="lgtp", bufs=2) as lgtp,
                tc.tile_pool(name="work", bufs=3) as sbw,
                tc.tile_pool(name="p2w", bufs=2) as p2w,
                tc.tile_pool(name="p2o", bufs=10) as p2o,
            ):
                # ---------- phase 2 work-item machinery ----------
                # Each item is (pe, act): `pe` emits PE/Pool/DMA work at the
                # top of a step (fills the LSTM-tail bubble), `act` emits
                # Activation work right after the step's softmax exp (fills
                # the exp->tanh gap in the in-order Act stream).

                def p2_items(tc_i, drain=False):
                    """Work items for t-chunk tc_i (32 rows). Chunk 3
                    overlaps chunk 2 by one row (t=95) so all four chunks
                    are uniformly 32 rows; the overlap row is recomputed
                    identically and double-written."""
                    ts = tc_i * 32 if tc_i < 3 else TT - 32
                    nt = 32
                    st = {}
                    items = []

                    def common_mms(pool, tag, j):
                        off, w = VCH[j]
                        ps = pool.tile([128, 512], F32, tag=tag, name=tag)
                        for k in range(2):
                            nc.tensor.matmul(
                                ps[:, 0:w],
                                linT_sb[:, k, tc_i, :, :],
                                cembt_sb[:, k, off:off + w],
                                start=(k == 0), stop=(k == 1),
                            )
                        return ps, w

                    def batched_mms(pool, tag):
                        ps = pool.tile([128, 512], F32, tag=tag, name=tag)
                        nmm = 2 + (1 if cfg["out_mask"] else 0)
                        for b in range(BS):
                            for k in range(2):
                                nc.tensor.matmul(
                                    ps[32 * b:32 * b + nt, 0:NB],
                                    linT_sb[:, k, tc_i, b, :],
                                    obt_sb[:, k, b, :],
                                    start=(k == 0), stop=(k == nmm - 1),
                                    tile_position=(0, 32 * b),
                                )
                            if cfg["out_mask"]:
                                nc.tensor.matmul(
                                    ps[32 * b:32 * b + nt, 0:NB],
                                    onest_sb[0:1, ts:ts + nt],
                                    bmr_sb[b:b + 1, :],
                                    start=False, stop=True,
                                    tile_position=(0, 32 * b),
                                )
                        return ps, NB

                    def mk_pass1(j):
                        cell = {}

                        def pe():
                            if j == 0 and not drain:
                                st["lgt"] = lgtp.tile(
                                    [128, NCH, 512], BF16, tag="lgt",
                                    name="lgt")
                            if j < len(VCH):
                                cell["x"], cell["w"] = common_mms(p2a, "x1", j)
                            else:
                                cell["x"], cell["w"] = batched_mms(p2a, "x1")
                            if not drain:
                                # stash logits in sbuf so pass 2 needs no
                                # PE/PSUM (lands in the DVE-idle window at
                                # step start); the drain recomputes instead
                                nc.vector.tensor_copy(
                                    st["lgt"][:, j, 0:cell["w"]],
                                    cell["x"][:, 0:cell["w"]],
                                )

                        def act():
                            if j == 0:
                                st["ss"] = p2w.tile(
                                    [128, NCH], F32, tag="ss", name="ss")
                                st["scr"] = p2w.tile(
                                    [128, 512], BF16, tag="scr", name="scr")
                            w = cell["w"]
                            nc.scalar.activation(
                                st["scr"][:, 0:w], cell.pop("x")[:, 0:w],
                                AF.Exp, accum_out=st["ss"][:, j:j + 1],
                            )
                        return (pe, act)

                    def mk_lse():
                        def act():
                            s_ = p2w.tile([128, 1], F32, tag="S", name="S")
                            nc.vector.reduce_sum(
                                s_[:], st["ss"][:], axis=mybir.AxisListType.X
                            )
                            # lse = ln(S) WITHOUT the Ln table (whose two
                            # 1283ns reloads stall phase-1 acts): Newton on
                            # y <- y + S*exp(-y) - 1 with the resident Exp
                            # table, seeded from the float exponent bits
                            # (seed err <= 0.35, 3 iterations -> <= 2e-6).
                            # Shift and arith must be separate ops; the
                            # -127 bias folds into the affine constant.
                            ei = p2w.tile([128, 1], mybir.dt.int32,
                                          tag="ei", name="ei")
                            nc.vector.tensor_scalar(
                                ei[:], s_[:].bitcast(mybir.dt.int32),
                                23, None, ALU.arith_shift_right)
                            ef = p2w.tile([128, 1], F32, tag="ef", name="ef")
                            nc.vector.tensor_copy(ef[:], ei[:])
                            ya = p2w.tile([128, 1], F32, tag="ya", name="ya")
                            yb = p2w.tile([128, 1], F32, tag="yb", name="yb")
                            nc.vector.tensor_scalar(
                                ya[:], ef[:], 0.6931472,
                                0.3466 - 127.0 * 0.6931472,
                                ALU.mult, ALU.add)
                            w_ = p2w.tile([128, 1], F32, tag="wn", name="wn")
                            tp = p2w.tile([128, 1], F32, tag="tp", name="tp")
                            cur, nxt = ya, yb
                            for _ in range(3):
                                nc.scalar.activation(
                                    w_[:], cur[:], AF.Exp, scale=-1.0)
                                nc.vector.tensor_tensor(
                                    tp[:], s_[:], w_[:], ALU.mult)
                                nc.vector.scalar_tensor_tensor(
                                    nxt[:], tp[:], 1.0, cur[:],
                                    ALU.subtract, ALU.add)
                                cur, nxt = nxt, cur
                            st["lse"] = cur
                        return (None, act)

                    def mk_pass2(j):
                        def pe():
                            if j < len(VCH):
                                voff, w = VCH[j]
                            else:
                                voff, w = NCOM, NB
                            ot = p2o.tile([128, 512], F32, tag="ot", name="ot")
                            if drain:
                                # loop is over: PE and DVE are idle, so
                                # recompute the logits and subtract straight
                                # from PSUM instead of staging via lgt/Pool
                                if j < len(VCH):
                                    ps, _ = common_mms(p2a, "x1", j)
                                else:
                                    ps, _ = batched_mms(p2a, "x1")
                                nc.vector.tensor_scalar(
                                    ot[:, 0:w], ps[:, 0:w],
                                    st["lse"][:], None, ALU.subtract,
                                )
                            else:
                                nc.gpsimd.tensor_scalar(
                                    ot[:, 0:w], st["lgt"][:, j, 0:w],
                                    st["lse"][:], None, ALU.subtract,
                                )
                            nc.sync.dma_start(
                                out[0:BS, ts:ts + nt, voff:voff + w],
                                ot[:, 0:w],
                            )
                        return (pe, None)

                    for j in range(NCH):
                        items.append(list(mk_pass1(j)) + ["p1"])
                    items.append(list(mk_lse()) + ["lse"])
                    for j in range(NCH):
                        items.append(list(mk_pass2(j)) + ["p2"])
                    return items

                p2queue = []

                def pop_p2(slot, budget=1):
                    """Emit up to `budget` parts for this slot, scanning
                    from the head. Slot-0 (pe) may run ahead of pending act
                    parts except a pending lse (whose Python closure creates
                    the tile that pass-2 subtracts read). Slot-1 (act) parts
                    run strictly in order, each after its own pe part."""
                    emitted = 0
                    i = 0
                    while i < len(p2queue) and emitted < budget:
                        it = p2queue[i]
                        if slot == 0:
                            if it[0] is not None:
                                f = it[0]
                                it[0] = None
                                f()
                                emitted += 1
                            if it[0] is None and it[1] is None:
                                p2queue.pop(i)
                                continue
                            if it[1] is not None and it[2] == "lse":
                                break
                            i += 1
                        else:
                            if it[1] is not None:
                                if it[0] is not None:
                                    break
                                f = it[1]
                                it[1] = None
                                f()
                                emitted += 1
                            if it[0] is None and it[1] is None:
                                p2queue.pop(i)
                                continue
                            i += 1

                # ---------- phase 1 steps ----------
                hT_cur = None
                cT_cur = None
                nb_ = 1 if cfg["any_bias"] else 0
                # merged per-step PSUM bank, f32 column map:
                #   scores sT(b,lc): 4b+lc        [0:16)
                #   Z bcast zb(b):   16+b         [16:20)
                #   attn at(dc,b):   20+4dc+b     [20:28)
                #   combine cb(hc,b):28+4hc+b     [28:36)
                #   gates gp(gc,b):  36+4gc+b     [36:68)
                #   lin lp(dc,b):    68+4dc+b     [68:76)
                for t in range(0 if cfg.get("skip_p1") else TT):
                    tg_, tr_ = t // 32, t % 32
                    if not cfg.get("skip_p2") and t >= 32 and t % 32 == 0:
                        p2queue.extend(p2_items(t // 32 - 1))
                    pp = sps.tile([128, 76], F32, tag="pp", name="pp")
                    zp = zps.tile([128, BS], F32, tag="zp", name="zp")

                    # -- phase-2 PE/Pool/DMA work: fills the LSTM-tail
                    #    bubble while this step waits for h --
                    pop_p2(0, 1)
                    # -- s0 inject (h-independent, off critical path) --
                    for b in range(BS):
                        for lc in range(4):
                            nc.tensor.matmul(
                                pp[:, 4 * b + lc:4 * b + lc + 1],
                                s032_sb[:, b, lc, tg_, :],
                                id32_sb[:, tr_:tr_ + 1],
                                start=True, stop=(t == 0),
                            )
                    # -- scores h-half (critical path: first after h lands) --
                    if t > 0:
                        for b in range(BS):
                            for lc in range(4):
                                for k in range(2):
                                    nc.tensor.matmul(
                                        pp[:, 4 * b + lc:4 * b + lc + 1],
                                        m2t_sb[:, k, b, lc * 128:(lc + 1) * 128],
                                        hT_cur[:, k, b:b + 1],
                                        start=False, stop=(k == 1),
                                    )
                    # -- softmax numerator --
                    ex = sbw.tile([128, BS, 4], BF16, tag="ex")
                    nc.scalar.activation(ex[:], pp[:, 0:16], AF.Exp)
                    # -- phase-2 Act work: fills the exp->tanh gap --
                    if not cfg.get("skip_p2"):
                        pop_p2(1, 1)
                    # -- Z (partition sum via all-ones stationary) --
                    for lc in range(4):
                        nc.tensor.matmul(
                            zp[:], ones_sb[:],
                            ex[:, :, lc],
                            start=(lc == 0), stop=(lc == 3),
                        )
                    # -- combine attn-half directly on the softmax numerator
                    #    (W2 folded through the encoder: C2v = EW^T @ ex =
                    #    W2 @ attn_raw, unnormalized) --
                    for b in range(BS):
                        for hc in range(2):
                            for lc in range(4):
                                nc.tensor.matmul(
                                    pp[:, 20 + 4 * hc + b:21 + 4 * hc + b],
                                    ew_sb[:, b, lc, hc * 128:(hc + 1) * 128],
                                    ex[:, b, lc:lc + 1],
                                    start=(lc == 0), stop=False,
                                )
                    # -- gates: bias + h-half (PE idles here anyway) --
                    if nb_:
                        for gc in range(8):
                            nc.tensor.matmul(
                                pp[:, 36 + 4 * gc:40 + 4 * gc],
                                brow_sb[0:1, 2 * H + gc * 128:2 * H + (gc + 1) * 128],
                                onesb_sb[0:1, :],
                                start=True, stop=False,
                            )
                    if t > 0:
                        for gc in range(8):
                            for kc in (2, 3):
                                nc.tensor.matmul(
                                    pp[:, 36 + 4 * gc:40 + 4 * gc],
                                    lwt_sb[:, kc, gc * 128:(gc + 1) * 128],
                                    hT_cur[:, kc - 2, :],
                                    start=(kc == 2 and not nb_), stop=False,
                                )
                    # -- combine prev-half scaled by Z (relu homogeneity:
                    #    comb = relu(u + v/Z) = (1/Z)*relu(W1@(Z*prev) + v));
                    #    sprev FIRST in the DVE queue (it gates the PE chain),
                    #    recip second (only needed by the final max-scale) --
                    sprev = sbw.tile([128, 2, BS], BF16, tag="sprev")
                    nc.vector.tensor_tensor(
                        sprev[:], tgtt_sb[:, :, t, :],
                        zp[:].unsqueeze(1).to_broadcast([128, 2, BS]),
                        ALU.mult,
                    )
                    rb = sbw.tile([128, BS], F32, tag="rb")
                    nc.vector.reciprocal(rb[:], zp[:])
                    if nb_:
                        zrow = sbw.tile([1, BS], BF16, tag="zrow")
                        nc.vector.tensor_copy(zrow[:], zp[0:1, :])
                        for hc in range(2):
                            nc.tensor.matmul(
                                pp[:, 20 + 4 * hc:24 + 4 * hc],
                                brow_sb[0:1, hc * 128:(hc + 1) * 128],
                                zrow[:],
                                start=False, stop=False,
                            )
                    for hc in range(2):
                        for kc in range(2):
                            nc.tensor.matmul(
                                pp[:, 20 + 4 * hc:24 + 4 * hc],
                                cwt_sb[:, kc, hc * 128:(hc + 1) * 128],
                                sprev[:, kc, :],
                                start=False, stop=(kc == 1),
                            )
                    cbT = sbw.tile([128, 2, BS], BF16, tag="cbT")
                    nc.vector.scalar_tensor_tensor(
                        cbT[:], pp[:, 20:28], 0.0,
                        rb[:].unsqueeze(1).to_broadcast([128, 2, BS]),
                        ALU.max, ALU.mult,
                    )
                    # -- gates comb-half + tanh --
                    for gc in range(8):
                        for kc in range(2):
                            nc.tensor.matmul(
                                pp[:, 36 + 4 * gc:40 + 4 * gc],
                                lwt_sb[:, kc, gc * 128:(gc + 1) * 128],
                                cbT[:, kc, :],
                                start=(kc == 0 and t == 0 and not nb_),
                                stop=(kc == 1),
                            )
                    tga = sbw.tile([128, 8, BS], F32, tag="tga")
                    nc.scalar.activation(tga[:], pp[:, 36:68], AF.Tanh)
                    # gate layout: gc 0,1=i  2,3=f  4,5=g  6,7=o
                    # p_i = (t_i + 1) * t_g ; p_f = (t_f + 1) * c
                    pi = sbw.tile([128, 2, BS], F32, tag="pi")
                    nc.vector.scalar_tensor_tensor(
                        pi[:], tga[:, 0:2, :], 1.0, tga[:, 4:6, :],
                        ALU.add, ALU.mult,
                    )
                    if t > 0:
                        pf = sbw.tile([128, 2, BS], F32, tag="pf")
                        nc.vector.scalar_tensor_tensor(
                            pf[:], tga[:, 2:4, :], 1.0, cT_cur[:],
                            ALU.add, ALU.mult,
                        )
                        s2c = sbw.tile([128, 2, BS], F32, tag="s2c")
                        nc.vector.tensor_add(s2c[:], pi[:], pf[:])
                    else:
                        s2c = pi
                    # tanh(c_new) = tanh(0.5 * s2c)
                    thc = sbw.tile([128, 2, BS], F32, tag="thc")
                    nc.scalar.activation(thc[:], s2c[:], AF.Tanh, scale=0.5)
                    # h2 = 2h = (t_o + 1) * tanh(c_new)  (consumers prescaled)
                    hT_new = stp.tile([128, 2, BS], BF16, tag="hstate")
                    nc.vector.scalar_tensor_tensor(
                        hT_new[:], tga[:, 6:8, :], 1.0, thc[:],
                        ALU.add, ALU.mult,
                    )
                    # true cell state for next step's p_f (off critical path)
                    cT_new = stp.tile([128, 2, BS], F32, tag="cstate")
                    nc.vector.tensor_scalar(
                        cT_new[:], s2c[:], 0.5, None, ALU.mult,
                    )
                    # -- output projection (off critical path) --
                    if nb_:
                        for dc in range(2):
                            nc.tensor.matmul(
                                pp[:, 68 + 4 * dc:72 + 4 * dc],
                                brow_sb[0:1, 6 * H + dc * 128:6 * H + (dc + 1) * 128],
                                onesb_sb[0:1, :],
                                start=True, stop=False,
                            )
                    for dc in range(2):
                        for k in range(2):
                            nc.tensor.matmul(
                                pp[:, 68 + 4 * dc:72 + 4 * dc],
                                owt_sb[:, k, dc * 128:(dc + 1) * 128],
                                hT_new[:, k, :],
                                start=(k == 0 and not nb_), stop=(k == 1),
                            )
                    if t < 95:
                        nc.vector.tensor_copy(
                            linT_sb[:, :, t // 32, :, t % 32], pp[:, 68:76])
                        if t == 64 - 1:
                            pass
                    if t == 95:
                        nc.vector.tensor_copy(
                            linT_sb[:, :, 2, :, 31], pp[:, 68:76])
                        nc.vector.tensor_copy(
                            linT_sb[:, :, 3, :, 0], pp[:, 68:76])
                    elif t > 95:
                        nc.vector.tensor_copy(
                            linT_sb[:, :, 3, :, t - 95], pp[:, 68:76])

                    if t == 0 and not cfg["any_bias"]:
                        # M2[b] = enc[b] @ M2W, [f-part, fc, b, l]; needed
                        # from step 1 on, so emitted at the tail of step 0
                        for b_ in range(BS):
                            for c_ in range(2):
                                psm = p2a.tile([128, 512], F32, tag="x1",
                                               name="psm")
                                for k_ in range(2):
                                    nc.tensor.matmul(
                                        psm[:],
                                        m2w_sb[:, k_, c_ * 128:(c_ + 1) * 128],
                                        enct_sb0[:, k_, b_, :],
                                        start=(k_ == 0), stop=(k_ == 1),
                                    )
                                if (b_ * 2 + c_) % 2 == 0:
                                    nc.vector.tensor_copy(
                                        m2t_sb[:, c_, b_, :], psm[:])
                                else:
                                    nc.scalar.activation(
                                        m2t_sb[:, c_, b_, :], psm[:],
                                        AF.Identity)

                    hT_cur, cT_cur = hT_new, cT_new

                # ---------- phase 2: drain remaining ----------
                if not cfg.get("skip_p2"):
                    for tc_i in (range(4) if cfg.get("skip_p1") else [3]):
                        p2queue.extend(p2_items(tc_i, drain=True))
                    while p2queue:
                        pop_p2(0, 3)
                        pop_p2(1, 3)

    nc.compile()
    return nc


_CACHE = {}


def kernel(**inputs):
    inp = {k: np.asarray(v) for k, v in inputs.items()}
    enc = inp["encoder_outputs"].astype(np.float32)
    encm = inp["encoder_outputs_mask"]
    ob = inp["output_batched_encodings"].astype(np.float32)
    obm = inp["output_batched_encodings_mask"]
    idx = inp["target_idxs"]
    cem = inp["common_embedding"].astype(np.float32)
    akw = inp["attn_key_w"].astype(np.float32)
    akb = inp["attn_key_b"].astype(np.float32)
    aqw = inp["attn_query_w"].astype(np.float32)
    aqb = inp["attn_query_b"].astype(np.float32)
    cw = inp["combine_w"].astype(np.float32)
    cb = inp["combine_b"].astype(np.float32)
    wih = inp["lstm_w_ih"].astype(np.float32)
    whh = inp["lstm_w_hh"].astype(np.float32)
    bih = inp["lstm_b_ih"].astype(np.float32)
    bhh = inp["lstm_b_hh"].astype(np.float32)
    ow = inp["out_w"].astype(np.float32)
    obias = inp["out_b"].astype(np.float32)

    # teacher-forced embedding gather (host: data-dependent indexing)
    is_c = idx < NCOM
    cidx = np.clip(idx, 0, NCOM - 1)
    bidx = np.clip(idx - NCOM, 0, NB - 1)
    ge_c = cem[cidx]                                   # (B, T, D)
    ge_b = np.take_along_axis(ob, bidx[..., None], axis=1)
    tgt = np.where(is_c[..., None], ge_c, ge_b)[:, :TT, :].astype(np.float32)

    any_bias = bool(
        np.any(akb) or np.any(aqb) or np.any(cb) or np.any(bih)
        or np.any(bhh) or np.any(obias)
    )
    enc_mask = not bool(encm.all())
    out_mask = not bool(obm.all())

    cfg = {"any_bias": any_bias, "enc_mask": enc_mask, "out_mask": out_mask}
    key = (any_bias, enc_mask, out_mask)
    if key not in _CACHE:
        _CACHE[key] = _build(cfg)
    nc = _CACHE[key]

    import ml_dtypes
    bft = ml_dtypes.bfloat16

    # scale folding:
    #   h is stored as 2h  -> every consumer of h gets x0.5:
    #     W_hh (lstm), akw_h (M2), out_w
    #   i,f,o gate pre-activations are halved (sigmoid = 0.5*tanh(x/2)+0.5)
    #     -> scale those *columns* of [W_ih; W_hh] and lstm bias by 0.5
    gsc = np.ones((4 * H,), np.float32) * 0.5
    gsc[2 * H:3 * H] = 1.0                       # g gate keeps full scale
    lw_full = np.concatenate([wih.T, whh.T * 0.5], axis=0) * gsc[None, :]
    lbias = (bih + bhh) * gsc

    shared = {
        "cembt": _pmajor(np.ascontiguousarray(cem.T), 2).astype(bft),
        "wket": _pmajor(np.ascontiguousarray(akw[:, :D].T), 2).astype(bft),
        "wqe": _pmajor(np.ascontiguousarray(aqw), 2).astype(bft),
        "m2w": _pmajor(aqw.T @ (akw[:, D:] * 0.5), 2).astype(bft),
        "cwt": _pmajor(np.ascontiguousarray(cw.T), 4).astype(bft),
        "w2d": _pmajor(np.ascontiguousarray(cw[:, D:].T), 2).astype(bft),
        "lwt": _pmajor(lw_full, 4).astype(bft),
        "owt": _pmajor(np.ascontiguousarray(ow.T) * 0.5, 2).astype(bft),
        "id32": np.eye(32, dtype=np.float32).astype(bft),
        "ones128": np.ones((128, 128), bft),
    }
    if any_bias:
        shared["wqt"] = _pmajor(np.ascontiguousarray(aqw.T), 2).astype(bft)
        shared["wkh2"] = _pmajor(
            np.ascontiguousarray(akw[:, D:]) * 0.5, 2).astype(bft)
        shared["bqc"] = _pmajor(aqb, 2)
        shared["bkc"] = _pmajor(akb, 2)
        shared["brow"] = np.concatenate(
            [cb, lbias, obias * 0.5]
        )[None, :].astype(bft)
        shared["onesb"] = np.ones((1, BS), bft)
    if out_mask:
        shared["onest"] = np.ones((1, TP), bft)

    in_maps = []
    for c in range(NCORES):
        sl = slice(c * BS, (c + 1) * BS)
        e = enc[sl]                                    # (BS, L, D)
        tg_ = tgt[sl]                                  # (BS, TT, D)
        obs = ob[sl]                                   # (BS, NB, D)
        m = dict(shared)
        m["enct"] = np.ascontiguousarray(
            e.transpose(2, 0, 1).reshape(2, 128, BS, L).transpose(1, 0, 2, 3)
        ).astype(bft)
        m["tgtt"] = np.ascontiguousarray(
            tg_.transpose(2, 1, 0).reshape(2, 128, TT, BS).transpose(1, 0, 2, 3)
        ).astype(bft)
        m["obt"] = np.ascontiguousarray(
            obs.transpose(2, 0, 1).reshape(2, 128, BS, NB).transpose(1, 0, 2, 3)
        ).astype(bft)
        if enc_mask:
            m["em_r"] = np.where(encm[sl], 0.0, -1e30).astype(bft)
            m["ones127"] = np.ones((1, TT), bft)
        if out_mask:
            m["bmr"] = np.where(obm[sl], 0.0, -1e30).astype(bft)
        in_maps.append(m)

    res = run_bass_kernel_spmd(nc, in_maps, list(range(NCORES)))
    outs = [res.results[c]["out"].reshape(BS, TT, V) for c in range(NCORES)]
    return np.concatenate(outs, axis=0).astype(np.float32)
